# revision 5
# baseline (speedup 1.0000x reference)
"""Trainium2 Bass kernel for the CNN_PHMM_VAE loss (profile-HMM forward + KLD).

v2: stacked-state restructure of the v1 kernel. Same math (linear-space
forward with per-batch prefix-product reparametrization, bf16 state,
periodic rescale events corrected exactly on the host), but the I- and
D-states live stacked in one [128, K+1] tile Z = [FI ; FD], so the two
state mults per column collapse into ONE [128, K] DVE op (DVE cost
depends only on free-dim length, so a [128,*] op costs the same as
[64,*]), and the I-state update runs on the otherwise-idle Pool engine:

  DVE per column:   U = cicd*Z | t = FM+U[lo] | t2 = t+U[hi]
                    FM'[1:] = t2*EM | FD' = scan(q, FM')        (5 ops)
  Pool per column:  u3 = gr*FI | FI' = FM+u3                    (2 ops)

Rescale events every RS columns use Act-engine accumulation (row sums of
FM and FI via activation accum_out) instead of DVE-side accum, so steady
columns never pay stt cost. The event factor r is applied via: FM-path
stt at the event column (folds into the EM mult), [r;1]-scaled stt on the
stacked U read one column later, and a DVE stt for the FI update that
column. Host corrects all applied factors exactly via the logged z.
"""
import sys

sys.path.insert(0, "/opt/trn_rl_repo")

import os

import numpy as np

B, L, K, E = 512, 256, 128, 16
L = int(os.environ.get("PHMM_L", L))  # internal: small-L perf probes only
REPEAT = int(os.environ.get("PHMM_REPEAT", 1))  # internal: perf probes only
NCORES = 8
BS = B // NCORES
Kp1 = K + 1
RS = int(os.environ.get("PHMM_RS", 8))
EVENTS = [l for l in range(RS, L, RS)]           # event columns (FM-path stt)
NEV = max(len(EVENTS), 1)
ACC_COLS = {l - 2 for l in EVENTS}               # accum z on the FMn op here
ACC_EV = {l - 2: i for i, l in enumerate(EVENTS)}
RCH_COLS = {l - 2: i for i, l in enumerate(EVENTS)}  # r-chain runs here
APPLY_COLS = {l + 1 for l in EVENTS}             # fold r into FI/FD reads here
# progressive em chunking: a small first chunk un-gates column 1 quickly
if L >= 32:
    CHUNK_COLS = [4, 12] + [16] * ((L - 16) // 16)
else:
    CHUNK_COLS = [L]
assert sum(CHUNK_COLS) == L
CHUNK_START = [sum(CHUNK_COLS[:j]) for j in range(len(CHUNK_COLS))]
NCHUNK = len(CHUNK_COLS)
COL_CHUNK = []  # column l-1 -> (chunk j, offset c)
for j, (s, n) in enumerate(zip(CHUNK_START, CHUNK_COLS)):
    for c in range(n):
        COL_CHUNK.append((j, c))

M2M, M2I, M2D, I2M, I2I, D2M, D2D = 0, 1, 2, 3, 4, 5, 6

_cache = {}


def _build_program(loopn=0):
    """loopn>0 wraps the column recurrence in a For_i hardware loop that
    re-executes the identical body loopn times (state re-derives from the
    fm0/z0 tiles each iteration). Used only by test.py's timing."""
    import contextlib

    import concourse.bacc as bacc
    import concourse.tile as tile
    from concourse import mybir

    f32 = mybir.dt.float32
    bf16 = mybir.dt.bfloat16
    Alu = mybir.AluOpType
    Act = mybir.ActivationFunctionType

    nc = bacc.Bacc("TRN2", target_bir_lowering=False, debug=False)

    em_d = nc.declare_dram_parameter("em", [BS, L * K], bf16, isOutput=False)
    cicd_d = nc.declare_dram_parameter("cicd", [2 * BS, Kp1], bf16, isOutput=False)
    grci_d = nc.declare_dram_parameter("grci", [BS, Kp1], bf16, isOutput=False)
    q_d = nc.declare_dram_parameter("q", [BS, Kp1], bf16, isOutput=False)
    fm0_d = nc.declare_dram_parameter("fm0", [BS, Kp1], bf16, isOutput=False)
    icb_d = nc.declare_dram_parameter("icb", [BS, 1], f32, isOutput=False)
    al_d = nc.declare_dram_parameter("al", [BS, 3], f32, isOutput=False)
    mus_d = nc.declare_dram_parameter("mus", [BS, E], f32, isOutput=False)
    lv_d = nc.declare_dram_parameter("lv", [BS, E], f32, isOutput=False)
    v_d = nc.declare_dram_parameter("outv", [BS, 1], f32, isOutput=True)
    z_d = nc.declare_dram_parameter("outz", [BS, NEV], f32, isOutput=True)
    kld_d = nc.declare_dram_parameter("outk", [BS, 1], f32, isOutput=True)

    with tile.TileContext(nc) as tc:
        with tc.tile_pool(name="p", bufs=1) as pool:
            def T(shape, tag, dt=bf16):
                return pool.tile(shape, dt, tag=tag, name=tag)

            em = [T([BS, CHUNK_COLS[j] * K], f"em{j}") for j in range(NCHUNK)]
            cicd = T([2 * BS, Kp1], "cicd")
            grci = T([BS, Kp1], "grci"); q = T([BS, Kp1], "q")
            icb = T([BS, 1], "icb", f32)
            al = T([BS, 3], "al", f32)
            mus_t = T([BS, E], "mus", f32); lv_t = T([BS, E], "lv", f32)

            fm0 = T([BS, Kp1], "fm0")
            z0 = T([2 * BS, Kp1], "z0")
            fm_ab = [T([BS, Kp1], "fma"), T([BS, Kp1], "fmb")]
            z_ab = [T([2 * BS, Kp1], "za"), T([2 * BS, Kp1], "zb")]
            u_ab = [T([2 * BS, Kp1], "ua"), T([2 * BS, Kp1], "ub")]
            # t lives in partitions 64:128 so the t2-op's two inputs (t and
            # the u2-half of U) share a start partition (walrus requires it)
            t_ab = [T([2 * BS, K], "ta"), T([2 * BS, K], "tb")]
            t2_ab = [T([BS, K], "t2a"), T([BS, K], "t2b")]
            fdk_t = T([BS, 1], "fdk", f32)
            u3_ab = [T([BS, Kp1], "u3a"), T([BS, Kp1], "u3b")]
            zbuf = T([BS, NEV], "zbuf", f32)
            zm_t = T([BS, 1], "zm", f32); zi_t = T([BS, 1], "zi", f32)
            zsc = T([BS, Kp1], "zsc")          # Act accum scratch dest
            zc_t = T([BS, 1], "zc", f32)
            r_t = T([BS, 1], "r", f32)
            rd_t = T([2 * BS, 1], "rd", f32)   # [r ; 1] stacked scalar
            w0_t = T([BS, 1], "w0", f32); w1_t = T([BS, 1], "w1", f32)
            v_t = T([BS, 1], "v", f32)
            m2_t = T([BS, E], "m2", f32); s1_t = T([BS, E], "s1", f32)
            ee_t = T([BS, E], "ee", f32); s2_t = T([BS, E], "s2", f32)
            red_t = T([BS, 1], "red", f32); kld_t = T([BS, 1], "kld", f32)

            # em chunk 0 + the tables column 1 needs go first (SP dispatches
            # serialize at ~0.6us each, so issue order sets column-1 latency)
            def em_dma(j):
                s = CHUNK_START[j] * K
                nc.sync.dma_start(
                    em[j][:], em_d[:, s:s + CHUNK_COLS[j] * K])

            em_dma(0)
            nc.sync.dma_start(q[:], q_d[:])
            nc.sync.dma_start(fm0[:], fm0_d[:])
            nc.sync.dma_start(cicd[:], cicd_d[:])
            nc.sync.dma_start(grci[:], grci_d[:])
            nc.sync.dma_start(icb[:], icb_d[:])
            em_dma(1); em_dma(2)
            nc.sync.dma_start(al[:], al_d[:])
            nc.sync.dma_start(mus_t[:], mus_d[:]); nc.sync.dma_start(lv_t[:], lv_d[:])
            for j in range(3, NCHUNK):
                em_dma(j)

            nc.vector.memset(z0[:], 0.0)
            for tl in z_ab + fm_ab:
                nc.vector.memset(tl[:], 0.0)
            nc.vector.memset(rd_t[BS:2 * BS, :], 1.0)

            # FD0 = scan over FM0, into the D-half of z0
            nc.vector.tensor_tensor_scan(
                out=z0[BS:2 * BS, 1:Kp1], data0=q[:, 1:Kp1], data1=fm0[:, 0:K],
                initial=0.0, op0=Alu.mult, op1=Alu.add)

            # KLD = -0.5 * sum(1 + lv - mus^2 - exp(lv)); emitted before the
            # column loop so it fills the DMA-gated startup and its output
            # DMA isn't serialized behind the end-of-loop readout
            nc.vector.tensor_tensor(m2_t[:], mus_t[:], mus_t[:], Alu.mult)
            nc.vector.tensor_tensor(s1_t[:], lv_t[:], m2_t[:], Alu.subtract)
            nc.scalar.activation(ee_t[:], lv_t[:], Act.Exp)
            nc.vector.tensor_tensor(s2_t[:], s1_t[:], ee_t[:], Alu.subtract)
            nc.vector.tensor_reduce(
                red_t[:], s2_t[:], axis=mybir.AxisListType.X, op=Alu.add)
            nc.scalar.activation(
                kld_t[:], red_t[:], Act.Copy,
                bias=-0.5 * E, scale=-0.5)
            nc.sync.dma_start(kld_d[:], kld_t[:])

            loop_ctx = (tc.For_i(0, loopn, 1, name="rep") if loopn > 0
                        else contextlib.nullcontext())
            with loop_ctx:
                for l0 in range(1, REPEAT * L + 1):
                    l = (l0 - 1) % L + 1
                    fm_p = fm0 if l == 1 else fm_ab[(l - 1) % 2]
                    z_p = z0 if l == 1 else z_ab[(l - 1) % 2]
                    fm_n = fm_ab[l % 2]
                    z_n = z_ab[l % 2]
                    u_t = u_ab[l % 2]
                    t_t = t_ab[l % 2]
                    t2_t = t2_ab[l % 2]
                    u3_t = u3_ab[l % 2]
                    j, c = COL_CHUNK[l - 1]
                    emsl = em[j][:, c * K:(c + 1) * K]

                    # M/D-path on DVE: U = [u1;u2] = [ci;cd]*[FI;FD], full
                    # K+1 wide so the u1-half also feeds the Pool I-path
                    if l in APPLY_COLS:
                        # fold r into the FI-half read; FD-half already scaled
                        nc.vector.scalar_tensor_tensor(
                            u_t[:], z_p[:], rd_t[:], cicd[:],
                            Alu.mult, Alu.mult)
                    else:
                        nc.vector.tensor_tensor(u_t[:], cicd[:], z_p[:],
                                                Alu.mult)

                    # Pool I-path: u3 = gr*FI = (gr/ci)*u1 — reading u1
                    # instead of FI keeps this on Pool even at apply columns
                    # (u1 already carries the event factor r there)
                    nc.gpsimd.tensor_tensor(u3_t[:], grci[:], u_t[0:BS, :],
                                            Alu.mult)
                    nc.gpsimd.tensor_tensor(z_n[0:BS, :], fm_p[:], u3_t[:],
                                            Alu.add)

                    nc.vector.tensor_tensor(t_t[BS:2 * BS, :],
                                            fm_p[:, 0:K],
                                            u_t[0:BS, 0:K], Alu.add)
                    nc.vector.tensor_tensor(t2_t[:], t_t[BS:2 * BS, :],
                                            u_t[BS:2 * BS, 0:K], Alu.add)
                    if l in EVENTS:
                        # apply r to the M-path now; scan propagates it to D
                        nc.vector.scalar_tensor_tensor(
                            fm_n[:, 1:Kp1], t2_t[:], r_t[:], emsl,
                            Alu.mult, Alu.mult)
                    elif l in ACC_COLS:
                        # zm = sum_k FMn from a free accum on the existing
                        # mult; zi (raw FI mass, needed so r never
                        # over-scales FI-dominated rows) via one Act copy
                        nc.vector.scalar_tensor_tensor(
                            fm_n[:, 1:Kp1], t2_t[:], 1.0, emsl,
                            Alu.mult, Alu.mult, accum_out=zm_t[:])
                    else:
                        nc.vector.tensor_tensor(fm_n[:, 1:Kp1], t2_t[:], emsl,
                                                Alu.mult)
                    nc.vector.tensor_tensor_scan(
                        out=z_n[BS:2 * BS, 1:Kp1], data0=q[:, 1:Kp1],
                        data1=fm_n[:, 0:K],
                        initial=0.0, op0=Alu.mult, op1=Alu.add)

                    if l in RCH_COLS:
                        # z = zm + zi; r ready before the event column's
                        # FM-path stt two columns later
                        ev = RCH_COLS[l]
                        nc.scalar.activation(zsc[:], z_p[0:BS, :], Act.Copy,
                                             accum_out=zi_t[:])
                        nc.scalar.activation(zbuf[:, ev:ev + 1], zm_t[:],
                                             Act.Identity, bias=zi_t[:],
                                             scale=1.0)
                        nc.scalar.mul(zc_t[:], zbuf[:, ev:ev + 1], icb[:])
                        nc.vector.reciprocal_approx_fast(r_t[:], zc_t[:])
                        nc.scalar.copy(rd_t[0:BS, :], r_t[:])

            fm_p = fm_ab[L % 2]
            z_p = z_ab[L % 2]
            # readout v = aM*FM[K] + aI*FI[K] + aD*FD[K]
            nc.vector.scalar_tensor_tensor(
                out=w0_t[:], in0=fm_p[:, K:Kp1], scalar=al[:, 0:1],
                in1=fm_p[:, K:Kp1], op0=Alu.mult, op1=Alu.bypass)
            nc.vector.scalar_tensor_tensor(
                out=w1_t[:], in0=z_p[0:BS, K:Kp1], scalar=al[:, 1:2],
                in1=w0_t[:], op0=Alu.mult, op1=Alu.add)
            nc.scalar.copy(fdk_t[:], z_p[BS:2 * BS, K:Kp1])
            nc.vector.scalar_tensor_tensor(
                out=v_t[:], in0=fdk_t[:], scalar=al[:, 2:3],
                in1=w1_t[:], op0=Alu.mult, op1=Alu.add)
            nc.sync.dma_start(v_d[:], v_t[:])
            nc.sync.dma_start(z_d[:], zbuf[:])

    nc.compile()
    return nc


def _precompute(batch_input, a, e_m):
    """Host precompute in fp64. Returns device tables + host corrections."""
    import ml_dtypes

    a = a.astype(np.float64)
    sM2M = np.exp(a[:, :, M2M]); sI2M = np.exp(a[:, :, I2M])
    sD2M = np.exp(a[:, :, D2M]); sM2I4 = 0.25 * np.exp(a[:, :, M2I])
    sI2I4 = 0.25 * np.exp(a[:, :, I2I]); sM2D = np.exp(a[:, :, M2D])
    sD2D = np.exp(a[:, :, D2D])
    Bn = a.shape[0]

    Dhat = np.ones((Bn, Kp1))
    Dhat[:, 1:] = sM2D[:, :-1] / sM2M[:, :-1]
    cI = (sI2M * sM2I4 / sM2M)[:, :K]
    cD = (sD2M * Dhat / sM2M)[:, :K]
    grow = sI2I4
    lq = np.zeros((Bn, Kp1))
    lq[:, 1:] = (a[:, :-1, D2D] + np.log(Dhat[:, :-1]) - np.log(Dhat[:, 1:])
                 - a[:, :-1, M2M])
    q = np.exp(lq); q[:, 0] = 0.0

    # per-batch anchor from the max drawup of the q-prefix walk
    pref = np.cumsum(lq, axis=1)
    runmin = np.minimum.accumulate(pref, axis=1)
    Qspread = np.max(pref - runmin, axis=1)
    lcD = np.log(cD).max(axis=1)
    headD = Qspread + np.maximum(lcD, 0.0)
    logCb = np.clip(86.0 - 42.0 - headD, -20.0, 42.0)
    Cb = np.exp(logCb)

    logPMK = a[:, :K, M2M].sum(axis=1)
    alphas = np.stack([sM2M[:, K], sI2M[:, K] * sM2I4[:, K],
                       sD2M[:, K] * Dhat[:, K]], axis=1)

    bi = np.arange(Bn)[:, None, None]
    ki = np.arange(K)[None, None, :]
    EM = np.exp(e_m.astype(np.float64)[bi, ki, batch_input[:, :, None]])  # (B,L,K)

    fm0 = np.zeros((Bn, Kp1))
    fm0[:, 0] = Cb

    # stacked per-core [ci ; cd] coefficient table, padded to K+1 columns
    # (ci[K] = 1 so u1[K] = FI[K] feeds the grci-based I-path; cd[K] = 0)
    ncores = Bn // BS
    cIp = np.concatenate([cI, np.ones((Bn, 1))], axis=1)
    cDp = np.concatenate([cD, np.zeros((Bn, 1))], axis=1)
    cicd = np.concatenate(
        [np.concatenate([cIp[c * BS:(c + 1) * BS], cDp[c * BS:(c + 1) * BS]])
         for c in range(ncores)])
    # grci: u3 = gr*FI computed as (gr/ci)*u1 from the U-op's u1-half
    bfq = lambda x: x.astype(ml_dtypes.bfloat16).astype(np.float64)
    grci = grow / bfq(cIp)

    bf = ml_dtypes.bfloat16
    f = np.float32
    tables = dict(
        em=EM.reshape(Bn, L * K).astype(bf), cicd=cicd.astype(bf),
        grci=grci.astype(bf), q=q.astype(bf),
        fm0=fm0.astype(bf), icb=(1.0 / Cb)[:, None].astype(f),
        al=alphas.astype(f),
    )
    corr = dict(logCb=logCb, logPMK=logPMK)
    return tables, corr


def _get_exec(loopn=0, repeat=None):
    """Build program + a cached jitted shard_map executor (one compile)."""
    global REPEAT
    if repeat is None:
        repeat = REPEAT
    ckey = ("exec", loopn, repeat)
    if ckey in _cache:
        return _cache[ckey]
    import jax
    from jax.sharding import Mesh, PartitionSpec
    from jax.experimental.shard_map import shard_map
    from concourse import mybir
    from concourse.bass2jax import (
        install_neuronx_cc_hook, _bass_exec_p, partition_id_tensor)

    rep_save, REPEAT = REPEAT, repeat
    try:
        nc = _build_program(loopn=loopn)
    finally:
        REPEAT = rep_save
    install_neuronx_cc_hook()

    pname = nc.partition_id_tensor.name if nc.partition_id_tensor else None
    in_names, out_names, out_avals, zero_shapes = [], [], [], []
    for alloc in nc.m.functions[0].allocations:
        if not isinstance(alloc, mybir.MemoryLocationSet):
            continue
        name = alloc.memorylocations[0].name
        if alloc.kind == "ExternalInput":
            if name != pname:
                in_names.append(name)
        elif alloc.kind == "ExternalOutput":
            shape = tuple(alloc.tensor_shape)
            dtype = mybir.dt.np(alloc.dtype)
            out_names.append(name)
            out_avals.append(jax.core.ShapedArray(shape, dtype))
            zero_shapes.append((shape, dtype))
    n_params = len(in_names)
    all_names = in_names + out_names
    if pname is not None:
        all_names = all_names + [pname]
    donate = tuple(range(n_params, n_params + len(out_names)))

    def _body(*args):
        operands = list(args)
        if pname is not None:
            operands.append(partition_id_tensor())
        outs = _bass_exec_p.bind(
            *operands, out_avals=tuple(out_avals), in_names=tuple(all_names),
            out_names=tuple(out_names), lowering_input_output_aliases=(),
            sim_require_finite=True, sim_require_nnan=True, nc=nc)
        return tuple(outs)

    devices = jax.devices()[:NCORES]
    mesh = Mesh(np.asarray(devices), ("core",))
    in_specs = (PartitionSpec("core"),) * (n_params + len(out_names))
    out_specs = (PartitionSpec("core"),) * len(out_names)
    sharded = jax.jit(
        shard_map(_body, mesh=mesh, in_specs=in_specs, out_specs=out_specs,
                  check_rep=False),
        donate_argnums=donate, keep_unused=True)
    _cache[ckey] = (sharded, in_names, out_names, out_avals, n_params)
    return _cache[ckey]


def _run_device(tables_full):
    """tables_full: dict name -> full [B, ...] array (cicd: [2B, ...]).
    Returns dict of outputs concatenated over cores as [B, ...]."""
    sharded, in_names, out_names, out_avals, n_params = _get_exec()
    ins = [np.ascontiguousarray(tables_full[n]) for n in in_names]
    zeros = [np.zeros((NCORES * a.shape[0], *a.shape[1:]), a.dtype)
             for a in out_avals]
    outs = sharded(*ins, *zeros)
    return {n: np.asarray(o) for n, o in zip(out_names, outs)}


def kernel(batch_input, transition_probs, emission_probs, mus, logvars):
    batch_input = np.asarray(batch_input).astype(np.int64)
    a = np.asarray(transition_probs, dtype=np.float32)
    e_m = np.asarray(emission_probs, dtype=np.float32)
    mus = np.asarray(mus, dtype=np.float32)
    logvars = np.asarray(logvars, dtype=np.float32)

    tables, corr = _precompute(batch_input, a, e_m)
    tables["mus"] = mus
    tables["lv"] = logvars

    out = _run_device(tables)
    v = out["outv"][:, 0]
    z = out["outz"]
    kld = out["outk"][:, 0]

    v64 = np.maximum(v.astype(np.float64), 1e-300)
    z64 = np.maximum(z.astype(np.float64), 1e-300)
    logCb = corr["logCb"]
    nll = -(np.log(v64) - logCb + (np.log(z64) - logCb[:, None]).sum(axis=1)
            + corr["logPMK"])
    loss = nll.mean() + kld.astype(np.float64).mean()
    return np.float32(loss)
